# revision 9
# baseline (speedup 1.0000x reference)
"""LIF spike kernel for Trainium2 (8 NeuronCores, batch-parallel).

Problem: x [64,128,56,56] f32; LIF recurrence in blocks of lif=8 steps along
H (dim=2):
    u   = tau*o + x_j
    out = u if u > vth else vth          == max(u, vth)
    o'  = 0 if u > vth else u            == (u <= vth) * u

Sharding: batch dim 64 -> 8 cores x 8 batches, no communication.
Per-core layout: C=128 on partitions, free dim = H*W per batch tile.
"""

import numpy as np

_B, _C, _H, _W = 64, 128, 56, 56
_NCORES = 8
_BS = _B // _NCORES          # batches per core
_LIF = 8
_NB = _H // _LIF             # 7 blocks per image

_CACHE = {}


def _build(tau: float, vth: float):
    import concourse.bacc as bacc
    import concourse.mybir as mybir
    from concourse.tile import TileContext

    f32 = mybir.dt.float32
    op = mybir.AluOpType

    # Bacc (not raw Bass): its compile() runs generate_event_semaphores,
    # which splits multi-waits into EventSemaphore instructions (TRN2 allows
    # only one sem wait per instruction).
    nc = bacc.Bacc("TRN2")
    x = nc.dram_tensor("x", [_BS, _C, _H, _W], f32, kind="ExternalInput")
    y = nc.dram_tensor("y", [_BS, _C, _H, _W], f32, kind="ExternalOutput")

    HW = _H * _W

    # Two constraints shape this kernel:
    # 1. walrus rejects any instruction with >1 sem wait;
    # 2. Tile has 8 DMAHW sem lanes -- the 9th+ DMA gets a lane-recycling
    #    wait on top of its data wait (= 2 waits -> codegen failure).
    # So: at most 8 DMAs total (batches processed in pairs), xt slots never
    # reused (DMA-in: 0 waits), ot written only by DVE (DMA-out: 1 RAW wait),
    # and the first DVE op of a pair touches only xt+tmp so the DMA-in wait
    # and the ot-slot WAR wait land on different instructions.
    BP = 2                       # batches per super-tile
    NS = _BS // BP               # 4 super-tiles -> 8 DMAs
    FD = BP * _NB * _W           # per-step free-dim elements
    with TileContext(nc) as tc:
        with (
            tc.tile_pool(name="xin", bufs=NS) as xp,
            tc.tile_pool(name="out", bufs=2) as outp,
            tc.tile_pool(name="tmp", bufs=2) as tp,
        ):
            for s in range(NS):
                xt = xp.tile([_C, BP * HW], f32, tag="xt")
                nc.sync.dma_start(
                    out=xt[:, :].rearrange("c (b hw) -> c b hw", hw=HW),
                    in_=x[s * BP:(s + 1) * BP].rearrange("b c h w -> c b (h w)"),
                )
                ot = outp.tile([_C, BP * HW], f32, tag="ot")
                u = tp.tile([_C, FD], f32, tag="u")
                o = tp.tile([_C, FD], f32, tag="o")

                # (b nb) merge is valid: b stride = NB*LIF*W = 7*8*56, nb
                # stride = LIF*W -> uniform [count 14, stride 448] + j offset.
                xv = xt[:, :].rearrange("c (q l w) -> c q l w", l=_LIF, w=_W)
                ov = ot[:, :].rearrange("c (q l w) -> c q l w", l=_LIF, w=_W)
                u3 = u[:, :].rearrange("c (q w) -> c q w", w=_W)
                o3 = o[:, :].rearrange("c (q w) -> c q w", w=_W)

                for j in range(_LIF):
                    xj = xv[:, :, j, :]
                    if j == 0:
                        uin = xj                       # u_0 = x_0 (o starts at 0)
                    else:
                        # u = (o * tau) + x_j
                        nc.vector.scalar_tensor_tensor(
                            out=u3, in0=o3, scalar=tau, in1=xj,
                            op0=op.mult, op1=op.add,
                        )
                        uin = u3
                    if j < _LIF - 1:
                        # o' = (u <= vth) * u
                        nc.vector.scalar_tensor_tensor(
                            out=o3, in0=uin, scalar=vth, in1=uin,
                            op0=op.is_le, op1=op.mult,
                        )
                    # out_j = max(u, vth)
                    nc.vector.tensor_scalar_max(out=ov[:, :, j, :], in0=uin, scalar1=vth)
                nc.sync.dma_start(
                    out=y[s * BP:(s + 1) * BP].rearrange("b c h w -> c b (h w)"),
                    in_=ot[:, :].rearrange("c (b hw) -> c b hw", hw=HW),
                )
    nc.finalize()   # runs Bacc.compile(): reg alloc + event-sem wait splitting
    return nc


def _get_nc(tau: float, vth: float):
    key = (tau, vth)
    if key not in _CACHE:
        _CACHE[key] = _build(tau, vth)
    return _CACHE[key]


def _run(x, tau, vth, **spmd_kwargs):
    from concourse.bass_utils import run_bass_kernel_spmd

    x = np.ascontiguousarray(np.asarray(x, dtype=np.float32))
    assert x.shape == (_B, _C, _H, _W), x.shape
    tau_f = float(np.asarray(tau).reshape(-1)[0])
    vth_f = float(np.asarray(vth).reshape(-1)[0])
    nc = _get_nc(tau_f, vth_f)
    shards = np.split(x, _NCORES, axis=0)
    in_maps = [{"x": np.ascontiguousarray(s)} for s in shards]
    res = run_bass_kernel_spmd(nc, in_maps, list(range(_NCORES)), **spmd_kwargs)
    out = np.concatenate([r["y"] for r in res.results], axis=0)
    return out, res


def kernel(x, tau, vth, lif, dim):
    assert int(np.asarray(lif)) == _LIF and int(np.asarray(dim)) == 2
    out, _ = _run(x, tau, vth)
    return out
